# revision 24
# baseline (speedup 1.0000x reference)
"""Trainium2 Bass kernel for the CustomGRU problem.

Reference semantics (fp32):
    z = sigmoid(x_t @ Wz_x + bz + h @ Wz_h)
    r = sigmoid(x_t @ Wr_x + br + h @ Wr_h)
    h~ = tanh(x_t @ Wh_x + bh + (r*h) @ Wh_h)
    h  = (1-z)*h + z*h~            (T=512 steps)
    out = h_T @ Wfc + bfc

Sharding: pure data parallel over batch (8192 -> 8 cores x 1024); the
time recurrence runs locally per core; the tiny weights are replicated.

Per-core design (H-major layout, B=1024 split into G=2 pipelined batch
groups of Bg=512). The per-step critical chain is cut to
    sigma -> rh -> MM_h -> tanh -> p -> MM_p -> sigma(t+1)
by splitting the next step's zr-matmul into psum-accumulating parts:
with h' = h + z*(h~ - h) = p - q',  p = z*h~,  q' = (z-1)*h,
    zr_preact(t+1) = Wzr_h @ p(t) + (-Wzr_h) @ q'(t) + Wzr_x @ x(t+1) + b
where the q'/x part (MM_qx, with sign-negated h-weights) runs as soon as
sigma(t) is done -- only the p part trails tanh.  Details:
  - One fused zr weight [41, 97]: r cols 0-32, zeros 33-63, z cols
    64-96, so psum [97, Bg] feeds a single sigmoid whose output leaves
    r at partition base 0 and z at base 64 (both legal DVE operand
    bases; a 33-row DVE access cannot straddle partition 64).
  - x is fused into the matmul contraction: staging tiles S hold NB=8
    step slots of [q'; x] (41 rows); q'(t) is written by DVE into the
    next slot, x is DMA'd blockwise into rows 33-40.  R staging holds
    [rh; x] at partitions 64-104 for the h~ matmul (array row strips
    2-3, so its stationary coexists with the zr weights in strips 0-1).
  - Elementwise ops: u = z-1 rides gpsimd (off-chain); rh = r*h,
    q' = u*h, p = z*h~, h' = p - q' are DVE tensor_tensor (2x_1p mode,
    ~327ns for [33,512] fp16; walrus requires equal operand start
    partitions for scalar_tensor_tensor, so no 3-operand fusion).
  - Engine budget per step (both groups): ACT 4x~720ns (the wall,
    ~88% busy on HW), DVE 8x~327ns, PE 6 matmuls + ldweights; chain
    ~2.9-3.2us/step -> measured 1.67ms total on HW (vs 2.66ms for the
    previous 12-matmul/step kernel).
"""

import sys

sys.path.insert(0, "/opt/trn_rl_repo")

from contextlib import ExitStack

import ml_dtypes  # noqa: F401  (registers bfloat16/float16 with numpy)
import numpy as np
import orjson

import concourse.bacc as bacc
import concourse.bass as bass  # noqa: F401
import concourse.tile as tile
from concourse import mybir
from concourse.bass_utils import run_bass_kernel_spmd

N_CORES = 8
I_IN = 8
H = 33
HOR = 24
KC = I_IN + H  # 41 contraction depth

AF = mybir.ActivationFunctionType
ALU = mybir.AluOpType
DT = mybir.dt

NB = 8  # steps per staging block


# --------------------------------------------------------------------------
# walrus in this container rejects CTRL (Drain) instructions carrying more
# than one sync wait; Tile's kernel-tail drain always has several. Split
# them at the serialized-JSON level (mutating the live module corrupts it).
def _split_multiwait_drains(raw: bytes, max_waits: int = 1) -> bytes:
    m = orjson.loads(raw)
    changed = False
    for f in m["functions"]:
        for bb in f["blocks"]:
            out = []
            for inst in bb["instructions"]:
                si = inst.get("sync_info")
                ow = (si or {}).get("on_wait") or []
                if inst.get("opcode") == "Drain" and len(ow) > max_waits:
                    head, tail = ow[:-max_waits], ow[-max_waits:]
                    for k, w in enumerate(head):
                        clone = dict(inst)
                        clone["name"] = f"{inst['name']}-sw{k}"
                        clone["sync_info"] = {"on_update": [], "on_wait": [w]}
                        out.append(clone)
                    inst = dict(inst)
                    inst["sync_info"] = {
                        "on_update": si.get("on_update") or [],
                        "on_wait": tail,
                    }
                    changed = True
                out.append(inst)
            bb["instructions"] = out
    return orjson.dumps(m) if changed else raw


def _install_bir_patch(nc):
    orig = nc.to_json_bytes
    nc.to_json_bytes = lambda: _split_multiwait_drains(orig())


# --------------------------------------------------------------------------
def build_gru_nc(B: int, T: int, finalize: bool = True, G: int = 2, repeat: int = 1):
    """Build the per-core Bass module (B = per-core batch)."""
    nc = bacc.Bacc("TRN2", target_bir_lowering=False, debug=False)
    f32 = DT.float32
    f16 = DT.float16
    Bg = B // G
    nblk = T // NB
    assert T % NB == 0 and B % G == 0

    xS = nc.dram_tensor(
        "xS", [G, nblk, I_IN, NB, Bg], f16, kind="ExternalInput"
    ).ap()
    w_rzn = nc.dram_tensor("w_rzn", [KC, 97], f16, kind="ExternalInput").ap()
    w_rzp = nc.dram_tensor("w_rzp", [H, 97], f16, kind="ExternalInput").ap()
    w_hh = nc.dram_tensor("w_hh", [KC, H], f16, kind="ExternalInput").ap()
    b_sig = nc.dram_tensor("b_sig", [97, 1], f32, kind="ExternalInput").ap()
    b_h = nc.dram_tensor("b_h", [H, 1], f32, kind="ExternalInput").ap()
    w_fc = nc.dram_tensor("w_fc", [H, HOR], f16, kind="ExternalInput").ap()
    b_fc = nc.dram_tensor("b_fc", [HOR, 1], f32, kind="ExternalInput").ap()
    y = nc.dram_tensor("y", [HOR, B], f32, kind="ExternalOutput").ap()

    with tile.TileContext(nc) as tc:
        with ExitStack() as ctx:
            consts = ctx.enter_context(tc.tile_pool(name="consts", bufs=1))
            sstage = ctx.enter_context(tc.tile_pool(name="sstage", bufs=2))
            rstage = ctx.enter_context(tc.tile_pool(name="rstage", bufs=2))
            hpool = ctx.enter_context(tc.tile_pool(name="hpool", bufs=3))
            acts = ctx.enter_context(tc.tile_pool(name="acts", bufs=3))
            psum_zr = ctx.enter_context(
                tc.tile_pool(name="psum_zr", bufs=2, space="PSUM")
            )
            psum_h = ctx.enter_context(
                tc.tile_pool(name="psum_h", bufs=2, space="PSUM")
            )

            # ---- constants ----
            wrzn_t = consts.tile([KC, 97], f16)
            wrzp_t = consts.tile([H, 97], f16)
            whh_t = consts.tile([64 + KC, H], f16)  # rows 64-104 used
            bsig_t = consts.tile([97, 1], f32)
            bh_t = consts.tile([H, 1], f32)
            wfc_t = consts.tile([H, HOR], f16)
            bfc_t = consts.tile([HOR, 1], f32)
            nc.sync.dma_start(wrzn_t[:], w_rzn[:])
            nc.sync.dma_start(wrzp_t[:], w_rzp[:])
            nc.sync.dma_start(whh_t[64 : 64 + KC, :], w_hh[:])
            nc.sync.dma_start(bsig_t[:], b_sig[:])
            nc.sync.dma_start(bh_t[:], b_h[:])
            nc.sync.dma_start(wfc_t[:], w_fc[:])
            nc.sync.dma_start(bfc_t[:], b_fc[:])

            # per-group staging tiles keyed by block parity, and state
            S_t = [[None, None] for _ in range(G)]
            R_t = [[None, None] for _ in range(G)]
            h_prev = [None] * G
            hfin = [None] * G
            pend = [None] * G  # (Ph, sig, q_ap) awaiting emit_back
            pzr_next = [None] * G  # psum tile accumulating step t+1's zr preact

            def block_prep(g, b, rep):
                """Allocate staging tiles for block b and DMA its x slice."""
                S = sstage.tile(
                    [KC, NB * Bg], f16, tag=f"s{g}", name=f"s{g}_{rep}_{b}"
                )
                R = rstage.tile(
                    [64 + KC, NB * Bg], f16, tag=f"r{g}", name=f"r{g}_{rep}_{b}"
                )
                src = xS[g, b]
                nc.sync.dma_start(
                    S[H:KC, :].rearrange("p (k b) -> p k b", b=Bg), src
                )
                nc.sync.dma_start(
                    R[64 + H : 64 + KC, :].rearrange("p (k b) -> p k b", b=Bg),
                    src,
                )
                S_t[g][b % 2] = S
                R_t[g][b % 2] = R
                return S

            def s_slot(g, t):
                S = S_t[g][(t // NB) % 2]
                j = t % NB
                return S[0:KC, j * Bg : (j + 1) * Bg]

            # chain-critical ops run in two half-batch lanes (Bh columns) so
            # each serial chain stage carries half the payload; off-chain ops
            # (u, q', MM_qx) stay full-width to bound the instruction count
            Bh = Bg // 2
            halves = ((0, Bh), (Bh, Bg))

            def emit_front(g, t, rep):
                j = t % NB
                R = R_t[g][(t // NB) % 2]
                c0 = j * Bg
                Pzr = pzr_next[g]
                h = h_prev[g]
                sig = acts.tile(
                    [97, Bg], f16, tag=f"sig{g}", name=f"sig{g}_{rep}_{t}"
                )
                Ph = psum_h.tile(
                    [H, Bg], f32, tag=f"ph{g}", name=f"ph{g}_{rep}_{t}"
                )
                for x0, x1 in halves:
                    nc.scalar.activation(
                        sig[0:97, x0:x1], Pzr[0:97, x0:x1], AF.Sigmoid,
                        bias=bsig_t[:],
                    )
                    # rh = r * h  ->  R rows 64-96 (upper-half DVE write)
                    nc.vector.tensor_mul(
                        R[64 : 64 + H, c0 + x0 : c0 + x1],
                        sig[0:H, x0:x1], h[0:H, x0:x1],
                    )
                    nc.tensor.matmul(
                        Ph[:, x0:x1], whh_t[64 : 64 + KC, :],
                        R[64 : 64 + KC, c0 + x0 : c0 + x1],
                        start=True, stop=True, skip_group_check=True,
                    )
                # u = z - 1 (gpsimd, off the critical chain)
                u = acts.tile([H, Bg], f16, tag=f"u{g}", name=f"u{g}_{rep}_{t}")
                nc.gpsimd.tensor_scalar(
                    u[:, :], sig[64 : 64 + H, :], 1.0, -1.0,
                    op0=ALU.mult, op1=ALU.add,
                )
                pend[g] = (Ph, sig, u, h)

            def emit_back(g, t, rep):
                Ph, sig, u, h = pend[g]
                # q' = u * h -> next step's S slot (rows 0-32)
                if t + 1 < T:
                    q_ap = s_slot(g, t + 1)[0:H, :]
                else:
                    qf = acts.tile([H, Bg], f16, tag=f"qf{g}", name=f"qf{g}_{rep}")
                    q_ap = qf[0:H, :]
                nc.vector.tensor_mul(q_ap, u[:, :], h[0:H, :])
                if t + 1 < T:
                    # open next step's zr accumulation with the q'/x part
                    P2 = psum_zr.tile(
                        [97, Bg], f32, tag=f"pzr{g}", name=f"pzr{g}_{rep}_{t + 1}"
                    )
                    nc.tensor.matmul(
                        P2[:, :], wrzn_t[:, :], s_slot(g, t + 1),
                        start=True, stop=False, skip_group_check=True,
                    )
                    pzr_next[g] = P2
                ht = acts.tile([97, Bg], f16, tag=f"ht{g}", name=f"ht{g}_{rep}_{t}")
                p = acts.tile([H, Bg], f16, tag=f"p{g}", name=f"p{g}_{rep}_{t}")
                if t + 1 < T:
                    hn = hpool.tile(
                        [H, Bg], f16, tag=f"h{g}", name=f"h{g}_{rep}_{t}"
                    )
                else:
                    hn = acts.tile([H, Bg], f16, tag=f"hf{g}", name=f"hf{g}_{rep}")
                    hfin[g] = hn
                for x0, x1 in halves:
                    nc.scalar.activation(
                        ht[64 : 64 + H, x0:x1], Ph[:, x0:x1], AF.Tanh,
                        bias=bh_t[:],
                    )
                    # p = z * h~   (both operands at base 64, out at base 0)
                    nc.vector.tensor_mul(
                        p[:, x0:x1], sig[64 : 64 + H, x0:x1],
                        ht[64 : 64 + H, x0:x1],
                    )
                    if t + 1 < T:
                        # close next step's zr accumulation with the p part
                        nc.tensor.matmul(
                            pzr_next[g][:, x0:x1], wrzp_t[:, :], p[:, x0:x1],
                            start=False, stop=True, skip_group_check=True,
                        )
                    # h' = p - q'
                    nc.vector.tensor_sub(
                        hn[:, x0:x1], p[:, x0:x1], q_ap[:, x0:x1]
                    )
                h_prev[g] = hn

            for rep in range(repeat):
                for g in range(G):
                    h0 = hpool.tile([H, Bg], f16, tag=f"h{g}", name=f"h0{g}_{rep}")
                    nc.vector.memset(h0[:, :], 0.0)
                    h_prev[g] = h0
                    S0 = block_prep(g, 0, rep)
                    nc.vector.memset(S0[0:H, 0:Bg], 0.0)  # q'(-1) = 0
                    P0 = psum_zr.tile(
                        [97, Bg], f32, tag=f"pzr{g}", name=f"pzr{g}_{rep}_0"
                    )
                    nc.tensor.matmul(
                        P0[:, :], wrzn_t[:, :], s_slot(g, 0),
                        start=True, stop=True, skip_group_check=True,
                    )
                    pzr_next[g] = P0
                for t in range(T):
                    if t % NB == 0:
                        b = t // NB
                        if b + 1 < nblk:
                            for g in range(G):
                                block_prep(g, b + 1, rep)
                    emit_front(0, t, rep)
                    emit_front(1, t, rep)
                    emit_back(0, t, rep)
                    emit_back(1, t, rep)

            # ---- final FC ----
            for g in range(G):
                pfc = psum_h.tile([HOR, Bg], f32, tag=f"ph{g}", name=f"pfc{g}")
                nc.tensor.matmul(
                    pfc[:, :], wfc_t[:], hfin[g][:, :],
                    start=True, stop=True, skip_group_check=True,
                )
                y_sb = acts.tile([HOR, Bg], f32, tag=f"sig{g}", name=f"ysb{g}")
                nc.scalar.activation(
                    y_sb[0:HOR, :], pfc[:, :], AF.Identity, bias=bfc_t[:]
                )
                nc.sync.dma_start(y[:, g * Bg : (g + 1) * Bg], y_sb[0:HOR, :])

    if finalize:
        nc.finalize()
        _install_bir_patch(nc)
    return nc


def prep_weights(Wz, bz, Wr, br, Wh, bh, Wfc, bfc):
    wrzp = np.zeros((H, 97), np.float32)
    wrzp[0:H, 0:H] = Wr[I_IN:]
    wrzp[0:H, 64 : 64 + H] = Wz[I_IN:]
    wrzn = np.zeros((KC, 97), np.float32)
    wrzn[0:H] = -wrzp
    wrzn[H:KC, 0:H] = Wr[:I_IN]
    wrzn[H:KC, 64 : 64 + H] = Wz[:I_IN]
    whh = np.zeros((KC, H), np.float32)
    whh[0:H] = Wh[I_IN:]
    whh[H:KC] = Wh[:I_IN]
    b_sig = np.zeros((97, 1), np.float32)
    b_sig[0:H, 0] = br
    b_sig[64 : 64 + H, 0] = bz
    return {
        "w_rzn": wrzn.astype(np.float16),
        "w_rzp": wrzp.astype(np.float16),
        "w_hh": whh.astype(np.float16),
        "b_sig": b_sig,
        "b_h": np.asarray(bh).reshape(H, 1).astype(np.float32),
        "w_fc": np.ascontiguousarray(Wfc).astype(np.float16),
        "b_fc": np.asarray(bfc).reshape(HOR, 1).astype(np.float32),
    }


def prep_x(xc, G):
    """[B, T, I] -> xS [G, T//NB, I, NB, Bg] fp16."""
    B, T, I = xc.shape
    Bg = B // G
    return np.ascontiguousarray(
        xc.reshape(G, Bg, T // NB, NB, I).transpose(0, 2, 4, 3, 1)
    ).astype(np.float16)


def run_gru(x, Wz, bz, Wr, br, Wh, bh, Wfc, bfc, n_cores=N_CORES, G=2,
            **spmd_kwargs):
    B_total, T, _ = x.shape
    B = B_total // n_cores
    nc = build_gru_nc(B, T, G=G)
    wmap = prep_weights(Wz, bz, Wr, br, Wh, bh, Wfc, bfc)
    in_maps = []
    for c in range(n_cores):
        xc = np.asarray(x[c * B : (c + 1) * B])
        in_maps.append({"xS": prep_x(xc, G), **wmap})
    res = run_bass_kernel_spmd(
        nc, in_maps, core_ids=list(range(n_cores)), **spmd_kwargs
    )
    y = np.concatenate(
        [res.results[c]["y"].T for c in range(n_cores)], axis=0
    ).astype(np.float32)
    return y, res


def kernel(x, Wz, bz, Wr, br, Wh, bh, Wfc, bfc):
    y, _ = run_gru(x, Wz, bz, Wr, br, Wh, bh, Wfc, bfc)
    return y


def make_inputs_for_timing(B, T, rng, G=2):
    """Random per-core input map matching build_gru_nc's dram tensors
    (timing only; values don't matter)."""
    Wz = rng.standard_normal((KC, H), dtype=np.float32) * 0.15
    Wr = rng.standard_normal((KC, H), dtype=np.float32) * 0.15
    Wh = rng.standard_normal((KC, H), dtype=np.float32) * 0.15
    Wfc = rng.standard_normal((H, HOR), dtype=np.float32) * 0.17
    wmap = prep_weights(
        Wz, rng.standard_normal(H, dtype=np.float32) * 0.15,
        Wr, rng.standard_normal(H, dtype=np.float32) * 0.15,
        Wh, rng.standard_normal(H, dtype=np.float32) * 0.15,
        Wfc, rng.standard_normal(HOR, dtype=np.float32) * 0.17,
    )
    xc = rng.standard_normal((B, T, I_IN), dtype=np.float32)
    return {"xS": prep_x(xc, G), **wmap}


# revision 25
# speedup vs baseline: 1.0359x; 1.0359x over previous
"""Trainium2 Bass kernel for the CustomGRU problem.

Reference semantics (fp32):
    z = sigmoid(x_t @ Wz_x + bz + h @ Wz_h)
    r = sigmoid(x_t @ Wr_x + br + h @ Wr_h)
    h~ = tanh(x_t @ Wh_x + bh + (r*h) @ Wh_h)
    h  = (1-z)*h + z*h~            (T=512 steps)
    out = h_T @ Wfc + bfc

Sharding: pure data parallel over batch (8192 -> 8 cores x 1024); the
time recurrence runs locally per core; the tiny weights are replicated.

Per-core design (H-major layout, B=1024 split into G=2 pipelined batch
groups of Bg=512). The per-step critical chain is cut to
    sigma -> rh -> MM_h -> tanh -> p -> MM_p -> sigma(t+1)
by splitting the next step's zr-matmul into psum-accumulating parts:
with h' = h + z*(h~ - h) = p - q',  p = z*h~,  q' = (z-1)*h,
    zr_preact(t+1) = Wzr_h @ p(t) + (-Wzr_h) @ q'(t) + Wzr_x @ x(t+1) + b
where the q'/x part (MM_qx, with sign-negated h-weights) runs as soon as
sigma(t) is done -- only the p part trails tanh.  Details:
  - One fused zr weight [41, 97]: r cols 0-32, zeros 33-63, z cols
    64-96, so psum [97, Bg] feeds a single sigmoid whose output leaves
    r at partition base 0 and z at base 64 (both legal DVE operand
    bases; a 33-row DVE access cannot straddle partition 64).
  - x is fused into the matmul contraction: staging tiles S hold NB=8
    step slots of [q'; x] (41 rows); q'(t) is written by DVE into the
    next slot, x is DMA'd blockwise into rows 33-40.  R staging holds
    [rh; x] at partitions 64-104 for the h~ matmul (array row strips
    2-3, so its stationary coexists with the zr weights in strips 0-1).
  - Elementwise ops: u = z-1 rides gpsimd (off-chain); rh = r*h,
    q' = u*h, p = z*h~, h' = p - q' are DVE tensor_tensor (2x_1p mode,
    ~327ns for [33,512] fp16; walrus requires equal operand start
    partitions for scalar_tensor_tensor, so no 3-operand fusion).
  - The step is chain-latency-bound on HW at ~4.75us (~720ns per ACT
    stage incl the non-overlapping 352-cycle pipe, ~450ns DVE and
    ~490ns MM stages, ~100ns per semaphore hop).  Splitting the chain
    into half-batch lanes saturates ACT (8 x (256+352)/1.2 > chain) and
    measured slower; more batch groups cannot shorten a serial
    recurrence and psum is at 8/8 banks.  Measured 2.43ms total
    (vs 2.68ms for the previous 12-matmul/step kernel, clean estimator).
"""

import sys

sys.path.insert(0, "/opt/trn_rl_repo")

from contextlib import ExitStack

import ml_dtypes  # noqa: F401  (registers bfloat16/float16 with numpy)
import numpy as np
import orjson

import concourse.bacc as bacc
import concourse.bass as bass  # noqa: F401
import concourse.tile as tile
from concourse import mybir
from concourse.bass_utils import run_bass_kernel_spmd

N_CORES = 8
I_IN = 8
H = 33
HOR = 24
KC = I_IN + H  # 41 contraction depth

AF = mybir.ActivationFunctionType
ALU = mybir.AluOpType
DT = mybir.dt

NB = 8  # steps per staging block


# --------------------------------------------------------------------------
# walrus in this container rejects CTRL (Drain) instructions carrying more
# than one sync wait; Tile's kernel-tail drain always has several. Split
# them at the serialized-JSON level (mutating the live module corrupts it).
def _split_multiwait_drains(raw: bytes, max_waits: int = 1) -> bytes:
    m = orjson.loads(raw)
    changed = False
    for f in m["functions"]:
        for bb in f["blocks"]:
            out = []
            for inst in bb["instructions"]:
                si = inst.get("sync_info")
                ow = (si or {}).get("on_wait") or []
                if inst.get("opcode") == "Drain" and len(ow) > max_waits:
                    head, tail = ow[:-max_waits], ow[-max_waits:]
                    for k, w in enumerate(head):
                        clone = dict(inst)
                        clone["name"] = f"{inst['name']}-sw{k}"
                        clone["sync_info"] = {"on_update": [], "on_wait": [w]}
                        out.append(clone)
                    inst = dict(inst)
                    inst["sync_info"] = {
                        "on_update": si.get("on_update") or [],
                        "on_wait": tail,
                    }
                    changed = True
                out.append(inst)
            bb["instructions"] = out
    return orjson.dumps(m) if changed else raw


def _install_bir_patch(nc):
    orig = nc.to_json_bytes
    nc.to_json_bytes = lambda: _split_multiwait_drains(orig())


# --------------------------------------------------------------------------
def build_gru_nc(B: int, T: int, finalize: bool = True, G: int = 2, repeat: int = 1):
    """Build the per-core Bass module (B = per-core batch)."""
    nc = bacc.Bacc("TRN2", target_bir_lowering=False, debug=False)
    f32 = DT.float32
    f16 = DT.float16
    Bg = B // G
    nblk = T // NB
    assert T % NB == 0 and B % G == 0

    xS = nc.dram_tensor(
        "xS", [G, nblk, I_IN, NB, Bg], f16, kind="ExternalInput"
    ).ap()
    w_rzn = nc.dram_tensor("w_rzn", [KC, 97], f16, kind="ExternalInput").ap()
    w_rzp = nc.dram_tensor("w_rzp", [H, 97], f16, kind="ExternalInput").ap()
    w_hh = nc.dram_tensor("w_hh", [KC, H], f16, kind="ExternalInput").ap()
    b_sig = nc.dram_tensor("b_sig", [97, 1], f32, kind="ExternalInput").ap()
    b_h = nc.dram_tensor("b_h", [H, 1], f32, kind="ExternalInput").ap()
    w_fc = nc.dram_tensor("w_fc", [H, HOR], f16, kind="ExternalInput").ap()
    b_fc = nc.dram_tensor("b_fc", [HOR, 1], f32, kind="ExternalInput").ap()
    y = nc.dram_tensor("y", [HOR, B], f32, kind="ExternalOutput").ap()

    with tile.TileContext(nc) as tc:
        with ExitStack() as ctx:
            consts = ctx.enter_context(tc.tile_pool(name="consts", bufs=1))
            sstage = ctx.enter_context(tc.tile_pool(name="sstage", bufs=2))
            rstage = ctx.enter_context(tc.tile_pool(name="rstage", bufs=2))
            hpool = ctx.enter_context(tc.tile_pool(name="hpool", bufs=3))
            acts = ctx.enter_context(tc.tile_pool(name="acts", bufs=3))
            psum_zr = ctx.enter_context(
                tc.tile_pool(name="psum_zr", bufs=2, space="PSUM")
            )
            psum_h = ctx.enter_context(
                tc.tile_pool(name="psum_h", bufs=2, space="PSUM")
            )

            # ---- constants ----
            wrzn_t = consts.tile([KC, 97], f16)
            wrzp_t = consts.tile([H, 97], f16)
            whh_t = consts.tile([64 + KC, H], f16)  # rows 64-104 used
            bsig_t = consts.tile([97, 1], f32)
            bh_t = consts.tile([H, 1], f32)
            wfc_t = consts.tile([H, HOR], f16)
            bfc_t = consts.tile([HOR, 1], f32)
            nc.sync.dma_start(wrzn_t[:], w_rzn[:])
            nc.sync.dma_start(wrzp_t[:], w_rzp[:])
            nc.sync.dma_start(whh_t[64 : 64 + KC, :], w_hh[:])
            nc.sync.dma_start(bsig_t[:], b_sig[:])
            nc.sync.dma_start(bh_t[:], b_h[:])
            nc.sync.dma_start(wfc_t[:], w_fc[:])
            nc.sync.dma_start(bfc_t[:], b_fc[:])

            # per-group staging tiles keyed by block parity, and state
            S_t = [[None, None] for _ in range(G)]
            R_t = [[None, None] for _ in range(G)]
            h_prev = [None] * G
            hfin = [None] * G
            pend = [None] * G  # (Ph, sig, q_ap) awaiting emit_back
            pzr_next = [None] * G  # psum tile accumulating step t+1's zr preact

            def block_prep(g, b, rep):
                """Allocate staging tiles for block b and DMA its x slice."""
                S = sstage.tile(
                    [KC, NB * Bg], f16, tag=f"s{g}", name=f"s{g}_{rep}_{b}"
                )
                R = rstage.tile(
                    [64 + KC, NB * Bg], f16, tag=f"r{g}", name=f"r{g}_{rep}_{b}"
                )
                src = xS[g, b]
                nc.sync.dma_start(
                    S[H:KC, :].rearrange("p (k b) -> p k b", b=Bg), src
                )
                nc.sync.dma_start(
                    R[64 + H : 64 + KC, :].rearrange("p (k b) -> p k b", b=Bg),
                    src,
                )
                S_t[g][b % 2] = S
                R_t[g][b % 2] = R
                return S

            def s_slot(g, t):
                S = S_t[g][(t // NB) % 2]
                j = t % NB
                return S[0:KC, j * Bg : (j + 1) * Bg]

            def emit_front(g, t, rep):
                j = t % NB
                R = R_t[g][(t // NB) % 2]
                c0, c1 = j * Bg, (j + 1) * Bg
                Pzr = pzr_next[g]
                sig = acts.tile(
                    [97, Bg], f16, tag=f"sig{g}", name=f"sig{g}_{rep}_{t}"
                )
                nc.scalar.activation(
                    sig[0:97, :], Pzr[0:97, :], AF.Sigmoid, bias=bsig_t[:]
                )
                h = h_prev[g]
                # rh = r * h  ->  R rows 64-96 (upper-half DVE write)
                nc.vector.tensor_mul(R[64 : 64 + H, c0:c1], sig[0:H, :], h[0:H, :])
                Ph = psum_h.tile(
                    [H, Bg], f32, tag=f"ph{g}", name=f"ph{g}_{rep}_{t}"
                )
                nc.tensor.matmul(
                    Ph[:, :], whh_t[64 : 64 + KC, :], R[64 : 64 + KC, c0:c1],
                    start=True, stop=True, skip_group_check=True,
                )
                # u = z - 1 (gpsimd, off the critical chain)
                u = acts.tile([H, Bg], f16, tag=f"u{g}", name=f"u{g}_{rep}_{t}")
                nc.gpsimd.tensor_scalar(
                    u[:, :], sig[64 : 64 + H, :], 1.0, -1.0,
                    op0=ALU.mult, op1=ALU.add,
                )
                pend[g] = (Ph, sig, u, h)

            def emit_back(g, t, rep):
                Ph, sig, u, h = pend[g]
                # q' = u * h -> next step's S slot (rows 0-32)
                if t + 1 < T:
                    q_ap = s_slot(g, t + 1)[0:H, :]
                else:
                    qf = acts.tile([H, Bg], f16, tag=f"qf{g}", name=f"qf{g}_{rep}")
                    q_ap = qf[0:H, :]
                nc.vector.tensor_mul(q_ap, u[:, :], h[0:H, :])
                if t + 1 < T:
                    # open next step's zr accumulation with the q'/x part
                    P2 = psum_zr.tile(
                        [97, Bg], f32, tag=f"pzr{g}", name=f"pzr{g}_{rep}_{t + 1}"
                    )
                    nc.tensor.matmul(
                        P2[:, :], wrzn_t[:, :], s_slot(g, t + 1),
                        start=True, stop=False, skip_group_check=True,
                    )
                    pzr_next[g] = P2
                ht = acts.tile([97, Bg], f16, tag=f"ht{g}", name=f"ht{g}_{rep}_{t}")
                nc.scalar.activation(
                    ht[64 : 64 + H, :], Ph[:, :], AF.Tanh, bias=bh_t[:]
                )
                # p = z * h~   (both operands at base 64, out at base 0)
                p = acts.tile([H, Bg], f16, tag=f"p{g}", name=f"p{g}_{rep}_{t}")
                nc.vector.tensor_mul(p[:, :], sig[64 : 64 + H, :], ht[64 : 64 + H, :])
                if t + 1 < T:
                    # close next step's zr accumulation with the p part
                    nc.tensor.matmul(
                        pzr_next[g][:, :], wrzp_t[:, :], p[:, :],
                        start=False, stop=True, skip_group_check=True,
                    )
                # h' = p - q'
                if t + 1 < T:
                    hn = hpool.tile(
                        [H, Bg], f16, tag=f"h{g}", name=f"h{g}_{rep}_{t}"
                    )
                else:
                    hn = acts.tile([H, Bg], f16, tag=f"hf{g}", name=f"hf{g}_{rep}")
                    hfin[g] = hn
                nc.vector.tensor_sub(hn[:, :], p[:, :], q_ap)
                h_prev[g] = hn

            for rep in range(repeat):
                for g in range(G):
                    h0 = hpool.tile([H, Bg], f16, tag=f"h{g}", name=f"h0{g}_{rep}")
                    nc.vector.memset(h0[:, :], 0.0)
                    h_prev[g] = h0
                    S0 = block_prep(g, 0, rep)
                    nc.vector.memset(S0[0:H, 0:Bg], 0.0)  # q'(-1) = 0
                    P0 = psum_zr.tile(
                        [97, Bg], f32, tag=f"pzr{g}", name=f"pzr{g}_{rep}_0"
                    )
                    nc.tensor.matmul(
                        P0[:, :], wrzn_t[:, :], s_slot(g, 0),
                        start=True, stop=True, skip_group_check=True,
                    )
                    pzr_next[g] = P0
                for t in range(T):
                    if t % NB == 0:
                        b = t // NB
                        if b + 1 < nblk:
                            for g in range(G):
                                block_prep(g, b + 1, rep)
                    emit_front(0, t, rep)
                    emit_front(1, t, rep)
                    emit_back(0, t, rep)
                    emit_back(1, t, rep)

            # ---- final FC ----
            for g in range(G):
                pfc = psum_h.tile([HOR, Bg], f32, tag=f"ph{g}", name=f"pfc{g}")
                nc.tensor.matmul(
                    pfc[:, :], wfc_t[:], hfin[g][:, :],
                    start=True, stop=True, skip_group_check=True,
                )
                y_sb = acts.tile([HOR, Bg], f32, tag=f"sig{g}", name=f"ysb{g}")
                nc.scalar.activation(
                    y_sb[0:HOR, :], pfc[:, :], AF.Identity, bias=bfc_t[:]
                )
                nc.sync.dma_start(y[:, g * Bg : (g + 1) * Bg], y_sb[0:HOR, :])

    if finalize:
        nc.finalize()
        _install_bir_patch(nc)
    return nc


def prep_weights(Wz, bz, Wr, br, Wh, bh, Wfc, bfc):
    wrzp = np.zeros((H, 97), np.float32)
    wrzp[0:H, 0:H] = Wr[I_IN:]
    wrzp[0:H, 64 : 64 + H] = Wz[I_IN:]
    wrzn = np.zeros((KC, 97), np.float32)
    wrzn[0:H] = -wrzp
    wrzn[H:KC, 0:H] = Wr[:I_IN]
    wrzn[H:KC, 64 : 64 + H] = Wz[:I_IN]
    whh = np.zeros((KC, H), np.float32)
    whh[0:H] = Wh[I_IN:]
    whh[H:KC] = Wh[:I_IN]
    b_sig = np.zeros((97, 1), np.float32)
    b_sig[0:H, 0] = br
    b_sig[64 : 64 + H, 0] = bz
    return {
        "w_rzn": wrzn.astype(np.float16),
        "w_rzp": wrzp.astype(np.float16),
        "w_hh": whh.astype(np.float16),
        "b_sig": b_sig,
        "b_h": np.asarray(bh).reshape(H, 1).astype(np.float32),
        "w_fc": np.ascontiguousarray(Wfc).astype(np.float16),
        "b_fc": np.asarray(bfc).reshape(HOR, 1).astype(np.float32),
    }


def prep_x(xc, G):
    """[B, T, I] -> xS [G, T//NB, I, NB, Bg] fp16."""
    B, T, I = xc.shape
    Bg = B // G
    return np.ascontiguousarray(
        xc.reshape(G, Bg, T // NB, NB, I).transpose(0, 2, 4, 3, 1)
    ).astype(np.float16)


def run_gru(x, Wz, bz, Wr, br, Wh, bh, Wfc, bfc, n_cores=N_CORES, G=2,
            **spmd_kwargs):
    B_total, T, _ = x.shape
    B = B_total // n_cores
    nc = build_gru_nc(B, T, G=G)
    wmap = prep_weights(Wz, bz, Wr, br, Wh, bh, Wfc, bfc)
    in_maps = []
    for c in range(n_cores):
        xc = np.asarray(x[c * B : (c + 1) * B])
        in_maps.append({"xS": prep_x(xc, G), **wmap})
    res = run_bass_kernel_spmd(
        nc, in_maps, core_ids=list(range(n_cores)), **spmd_kwargs
    )
    y = np.concatenate(
        [res.results[c]["y"].T for c in range(n_cores)], axis=0
    ).astype(np.float32)
    return y, res


def kernel(x, Wz, bz, Wr, br, Wh, bh, Wfc, bfc):
    y, _ = run_gru(x, Wz, bz, Wr, br, Wh, bh, Wfc, bfc)
    return y


def make_inputs_for_timing(B, T, rng, G=2):
    """Random per-core input map matching build_gru_nc's dram tensors
    (timing only; values don't matter)."""
    Wz = rng.standard_normal((KC, H), dtype=np.float32) * 0.15
    Wr = rng.standard_normal((KC, H), dtype=np.float32) * 0.15
    Wh = rng.standard_normal((KC, H), dtype=np.float32) * 0.15
    Wfc = rng.standard_normal((H, HOR), dtype=np.float32) * 0.17
    wmap = prep_weights(
        Wz, rng.standard_normal(H, dtype=np.float32) * 0.15,
        Wr, rng.standard_normal(H, dtype=np.float32) * 0.15,
        Wh, rng.standard_normal(H, dtype=np.float32) * 0.15,
        Wfc, rng.standard_normal(HOR, dtype=np.float32) * 0.17,
    )
    xc = rng.standard_normal((B, T, I_IN), dtype=np.float32)
    return {"xS": prep_x(xc, G), **wmap}


# revision 28
# speedup vs baseline: 1.0634x; 1.0266x over previous
"""Trainium2 Bass kernel for the CustomGRU problem.

Reference semantics (fp32):
    z = sigmoid(x_t @ Wz_x + bz + h @ Wz_h)
    r = sigmoid(x_t @ Wr_x + br + h @ Wr_h)
    h~ = tanh(x_t @ Wh_x + bh + (r*h) @ Wh_h)
    h  = (1-z)*h + z*h~            (T=512 steps)
    out = h_T @ Wfc + bfc

Sharding: pure data parallel over batch (8192 -> 8 cores x 1024); the
time recurrence runs locally per core; the tiny weights are replicated.

Per-core design (H-major layout, B=1024 split into G=2 pipelined batch
groups of Bg=512). The per-step critical chain is cut to
    sigma -> rh -> MM_h -> tanh -> p -> MM_p -> sigma(t+1)
by splitting the next step's zr-matmul into psum-accumulating parts:
with h' = h + z*(h~ - h) = p - q',  p = z*h~,  q' = (z-1)*h,
    zr_preact(t+1) = Wzr_h @ p(t) + (-Wzr_h) @ q'(t) + Wzr_x @ x(t+1) + b
where the q'/x part (MM_qx, with sign-negated h-weights) runs as soon as
sigma(t) is done -- only the p part trails tanh.  Details:
  - One fused zr weight [41, 97]: r cols 0-32, zeros 33-63, z cols
    64-96, so psum [97, Bg] feeds a single sigmoid whose output leaves
    r at partition base 0 and z at base 64 (both legal DVE operand
    bases; a 33-row DVE access cannot straddle partition 64).
  - x is fused into the matmul contraction: staging tiles S hold NB=8
    step slots of [q'; x] (41 rows); q'(t) is written by DVE into the
    next slot, x is DMA'd blockwise into rows 33-40.  R staging holds
    [rh; x] at partitions 64-104 for the h~ matmul (array row strips
    2-3, so its stationary coexists with the zr weights in strips 0-1).
  - Elementwise ops: u = z-1 rides gpsimd (off-chain); rh = r*h,
    q' = u*h, p = z*h~, h' = p - q' are DVE tensor_tensor (2x_1p mode,
    ~327ns for [33,512] fp16; walrus requires equal operand start
    partitions for scalar_tensor_tensor, so no 3-operand fusion).
  - The step is chain-latency-bound on HW at ~4.75us (~720ns per ACT
    stage incl the non-overlapping 352-cycle pipe, ~450ns DVE and
    ~490ns MM stages, ~100ns per semaphore hop).  Splitting the chain
    into half-batch lanes saturates ACT (8 x (256+352)/1.2 > chain) and
    measured slower; more batch groups cannot shorten a serial
    recurrence and psum is at 8/8 banks.  Measured 2.43ms total
    (vs 2.68ms for the previous 12-matmul/step kernel, clean estimator).
"""

import sys

sys.path.insert(0, "/opt/trn_rl_repo")

from contextlib import ExitStack

import ml_dtypes  # noqa: F401  (registers bfloat16/float16 with numpy)
import numpy as np
import orjson

import concourse.bacc as bacc
import concourse.bass as bass  # noqa: F401
import concourse.tile as tile
from concourse import mybir
from concourse.bass_utils import run_bass_kernel_spmd

N_CORES = 8
I_IN = 8
H = 33
HOR = 24
KC = I_IN + H  # 41 contraction depth

AF = mybir.ActivationFunctionType
ALU = mybir.AluOpType
DT = mybir.dt

NB = 8  # steps per staging block


# --------------------------------------------------------------------------
# walrus in this container rejects CTRL (Drain) instructions carrying more
# than one sync wait; Tile's kernel-tail drain always has several. Split
# them at the serialized-JSON level (mutating the live module corrupts it).
def _split_multiwait_drains(raw: bytes, max_waits: int = 1) -> bytes:
    m = orjson.loads(raw)
    changed = False
    for f in m["functions"]:
        for bb in f["blocks"]:
            out = []
            for inst in bb["instructions"]:
                si = inst.get("sync_info")
                ow = (si or {}).get("on_wait") or []
                if inst.get("opcode") == "Drain" and len(ow) > max_waits:
                    head, tail = ow[:-max_waits], ow[-max_waits:]
                    for k, w in enumerate(head):
                        clone = dict(inst)
                        clone["name"] = f"{inst['name']}-sw{k}"
                        clone["sync_info"] = {"on_update": [], "on_wait": [w]}
                        out.append(clone)
                    inst = dict(inst)
                    inst["sync_info"] = {
                        "on_update": si.get("on_update") or [],
                        "on_wait": tail,
                    }
                    changed = True
                out.append(inst)
            bb["instructions"] = out
    return orjson.dumps(m) if changed else raw


def _install_bir_patch(nc):
    orig = nc.to_json_bytes
    nc.to_json_bytes = lambda: _split_multiwait_drains(orig())


# --------------------------------------------------------------------------
def group_width(B: int, G: int) -> int:
    """Per-group batch width, rounded up to even (batch is padded to G*Bg)."""
    Bg = -(-B // G)
    return Bg + (Bg % 2)


def build_gru_nc(B: int, T: int, finalize: bool = True, G: int = 3, repeat: int = 1):
    """Build the per-core Bass module (B = per-core batch, padded to G*Bg)."""
    nc = bacc.Bacc("TRN2", target_bir_lowering=False, debug=False)
    f32 = DT.float32
    f16 = DT.float16
    Bg = group_width(B, G)
    BP = G * Bg
    nblk = T // NB
    assert T % NB == 0

    xS = nc.dram_tensor(
        "xS", [G, nblk, I_IN, NB, Bg], f16, kind="ExternalInput"
    ).ap()
    w_rzn = nc.dram_tensor("w_rzn", [KC, 97], f16, kind="ExternalInput").ap()
    w_rzp = nc.dram_tensor("w_rzp", [H, 97], f16, kind="ExternalInput").ap()
    w_hh = nc.dram_tensor("w_hh", [KC, H], f16, kind="ExternalInput").ap()
    b_sig = nc.dram_tensor("b_sig", [97, 1], f32, kind="ExternalInput").ap()
    b_h = nc.dram_tensor("b_h", [H, 1], f32, kind="ExternalInput").ap()
    w_fc = nc.dram_tensor("w_fc", [H, HOR], f16, kind="ExternalInput").ap()
    b_fc = nc.dram_tensor("b_fc", [HOR, 1], f32, kind="ExternalInput").ap()
    y = nc.dram_tensor("y", [HOR, BP], f32, kind="ExternalOutput").ap()

    with tile.TileContext(nc) as tc:
        with ExitStack() as ctx:
            consts = ctx.enter_context(tc.tile_pool(name="consts", bufs=1))
            sstage = ctx.enter_context(tc.tile_pool(name="sstage", bufs=2))
            rstage = ctx.enter_context(tc.tile_pool(name="rstage", bufs=2))
            hpool = ctx.enter_context(tc.tile_pool(name="hpool", bufs=3))
            acts = ctx.enter_context(tc.tile_pool(name="acts", bufs=3))
            psum_zr = ctx.enter_context(
                tc.tile_pool(name="psum_zr", bufs=1, space="PSUM")
            )
            psum_h = ctx.enter_context(
                tc.tile_pool(name="psum_h", bufs=1, space="PSUM")
            )

            # ---- constants ----
            wrzn_t = consts.tile([KC, 97], f16)
            wrzp_t = consts.tile([H, 97], f16)
            whh_t = consts.tile([64 + KC, H], f16)  # rows 64-104 used
            bsig_t = consts.tile([97, 1], f32)
            bh_t = consts.tile([H, 1], f32)
            wfc_t = consts.tile([H, HOR], f16)
            bfc_t = consts.tile([HOR, 1], f32)
            nc.sync.dma_start(wrzn_t[:], w_rzn[:])
            nc.sync.dma_start(wrzp_t[:], w_rzp[:])
            nc.sync.dma_start(whh_t[64 : 64 + KC, :], w_hh[:])
            nc.sync.dma_start(bsig_t[:], b_sig[:])
            nc.sync.dma_start(bh_t[:], b_h[:])
            nc.sync.dma_start(wfc_t[:], w_fc[:])
            nc.sync.dma_start(bfc_t[:], b_fc[:])

            # per-group staging tiles keyed by block parity, and state
            S_t = [[None, None] for _ in range(G)]
            R_t = [[None, None] for _ in range(G)]
            h_prev = [None] * G
            hfin = [None] * G
            pend = [None] * G  # (Ph, sig, q_ap) awaiting emit_back
            pzr_next = [None] * G  # psum tile accumulating step t+1's zr preact

            def block_prep(g, b, rep):
                """Allocate staging tiles for block b and DMA its x slice."""
                S = sstage.tile(
                    [KC, NB * Bg], f16, tag=f"s{g}", name=f"s{g}_{rep}_{b}"
                )
                R = rstage.tile(
                    [64 + KC, NB * Bg], f16, tag=f"r{g}", name=f"r{g}_{rep}_{b}"
                )
                src = xS[g, b]
                nc.sync.dma_start(
                    S[H:KC, :].rearrange("p (k b) -> p k b", b=Bg), src
                )
                nc.sync.dma_start(
                    R[64 + H : 64 + KC, :].rearrange("p (k b) -> p k b", b=Bg),
                    src,
                )
                S_t[g][b % 2] = S
                R_t[g][b % 2] = R
                return S

            def s_slot(g, t):
                S = S_t[g][(t // NB) % 2]
                j = t % NB
                return S[0:KC, j * Bg : (j + 1) * Bg]

            def emit_front(g, t, rep):
                j = t % NB
                R = R_t[g][(t // NB) % 2]
                c0, c1 = j * Bg, (j + 1) * Bg
                Pzr = pzr_next[g]
                sig = acts.tile(
                    [97, Bg], f16, tag=f"sig{g}", name=f"sig{g}_{rep}_{t}"
                )
                nc.scalar.activation(
                    sig[0:97, :], Pzr[0:97, :], AF.Sigmoid, bias=bsig_t[:]
                )
                h = h_prev[g]
                # rh = r * h  ->  R rows 64-96 (upper-half DVE write)
                nc.vector.tensor_mul(R[64 : 64 + H, c0:c1], sig[0:H, :], h[0:H, :])
                Ph = psum_h.tile(
                    [H, Bg], f32, tag=f"ph{g}", name=f"ph{g}_{rep}_{t}"
                )
                nc.tensor.matmul(
                    Ph[:, :], whh_t[64 : 64 + KC, :], R[64 : 64 + KC, c0:c1],
                    start=True, stop=True, skip_group_check=True,
                )
                # u = z - 1 (gpsimd, off the critical chain)
                u = acts.tile([H, Bg], f16, tag=f"u{g}", name=f"u{g}_{rep}_{t}")
                nc.gpsimd.tensor_scalar(
                    u[:, :], sig[64 : 64 + H, :], 1.0, -1.0,
                    op0=ALU.mult, op1=ALU.add,
                )
                pend[g] = (Ph, sig, u, h)

            def emit_back(g, t, rep):
                Ph, sig, u, h = pend[g]
                # q' = u * h -> next step's S slot (rows 0-32)
                if t + 1 < T:
                    q_ap = s_slot(g, t + 1)[0:H, :]
                else:
                    qf = acts.tile([H, Bg], f16, tag=f"qf{g}", name=f"qf{g}_{rep}")
                    q_ap = qf[0:H, :]
                nc.vector.tensor_mul(q_ap, u[:, :], h[0:H, :])
                if t + 1 < T:
                    # open next step's zr accumulation with the q'/x part
                    P2 = psum_zr.tile(
                        [97, Bg], f32, tag=f"pzr{g}", name=f"pzr{g}_{rep}_{t + 1}"
                    )
                    nc.tensor.matmul(
                        P2[:, :], wrzn_t[:, :], s_slot(g, t + 1),
                        start=True, stop=False, skip_group_check=True,
                    )
                    pzr_next[g] = P2
                ht = acts.tile([97, Bg], f16, tag=f"ht{g}", name=f"ht{g}_{rep}_{t}")
                nc.scalar.activation(
                    ht[64 : 64 + H, :], Ph[:, :], AF.Tanh, bias=bh_t[:]
                )
                # p = z * h~   (both operands at base 64, out at base 0)
                p = acts.tile([H, Bg], f16, tag=f"p{g}", name=f"p{g}_{rep}_{t}")
                nc.vector.tensor_mul(p[:, :], sig[64 : 64 + H, :], ht[64 : 64 + H, :])
                if t + 1 < T:
                    # close next step's zr accumulation with the p part
                    nc.tensor.matmul(
                        pzr_next[g][:, :], wrzp_t[:, :], p[:, :],
                        start=False, stop=True, skip_group_check=True,
                    )
                # h' = p - q'
                if t + 1 < T:
                    hn = hpool.tile(
                        [H, Bg], f16, tag=f"h{g}", name=f"h{g}_{rep}_{t}"
                    )
                else:
                    hn = acts.tile([H, Bg], f16, tag=f"hf{g}", name=f"hf{g}_{rep}")
                    hfin[g] = hn
                nc.vector.tensor_sub(hn[:, :], p[:, :], q_ap)
                h_prev[g] = hn

            for rep in range(repeat):
                for g in range(G):
                    h0 = hpool.tile([H, Bg], f16, tag=f"h{g}", name=f"h0{g}_{rep}")
                    nc.vector.memset(h0[:, :], 0.0)
                    h_prev[g] = h0
                    S0 = block_prep(g, 0, rep)
                    nc.vector.memset(S0[0:H, 0:Bg], 0.0)  # q'(-1) = 0
                    P0 = psum_zr.tile(
                        [97, Bg], f32, tag=f"pzr{g}", name=f"pzr{g}_{rep}_0"
                    )
                    nc.tensor.matmul(
                        P0[:, :], wrzn_t[:, :], s_slot(g, 0),
                        start=True, stop=True, skip_group_check=True,
                    )
                    pzr_next[g] = P0
                for t in range(T):
                    if t % NB == 0:
                        b = t // NB
                        if b + 1 < nblk:
                            for g in range(G):
                                block_prep(g, b + 1, rep)
                    for g in range(G):
                        emit_front(g, t, rep)
                    for g in range(G):
                        emit_back(g, t, rep)

            # ---- final FC ----
            for g in range(G):
                pfc = psum_h.tile([HOR, Bg], f32, tag=f"ph{g}", name=f"pfc{g}")
                nc.tensor.matmul(
                    pfc[:, :], wfc_t[:], hfin[g][:, :],
                    start=True, stop=True, skip_group_check=True,
                )
                y_sb = acts.tile([HOR, Bg], f32, tag=f"sig{g}", name=f"ysb{g}")
                nc.scalar.activation(
                    y_sb[0:HOR, :], pfc[:, :], AF.Identity, bias=bfc_t[:]
                )
                nc.sync.dma_start(y[:, g * Bg : (g + 1) * Bg], y_sb[0:HOR, :])

    if finalize:
        nc.finalize()
        _install_bir_patch(nc)
    return nc


def prep_weights(Wz, bz, Wr, br, Wh, bh, Wfc, bfc):
    wrzp = np.zeros((H, 97), np.float32)
    wrzp[0:H, 0:H] = Wr[I_IN:]
    wrzp[0:H, 64 : 64 + H] = Wz[I_IN:]
    wrzn = np.zeros((KC, 97), np.float32)
    wrzn[0:H] = -wrzp
    wrzn[H:KC, 0:H] = Wr[:I_IN]
    wrzn[H:KC, 64 : 64 + H] = Wz[:I_IN]
    whh = np.zeros((KC, H), np.float32)
    whh[0:H] = Wh[I_IN:]
    whh[H:KC] = Wh[:I_IN]
    b_sig = np.zeros((97, 1), np.float32)
    b_sig[0:H, 0] = br
    b_sig[64 : 64 + H, 0] = bz
    return {
        "w_rzn": wrzn.astype(np.float16),
        "w_rzp": wrzp.astype(np.float16),
        "w_hh": whh.astype(np.float16),
        "b_sig": b_sig,
        "b_h": np.asarray(bh).reshape(H, 1).astype(np.float32),
        "w_fc": np.ascontiguousarray(Wfc).astype(np.float16),
        "b_fc": np.asarray(bfc).reshape(HOR, 1).astype(np.float32),
    }


def prep_x(xc, G):
    """[B, T, I] -> xS [G, T//NB, I, NB, Bg] fp16, batch zero-padded to G*Bg."""
    B, T, I = xc.shape
    Bg = group_width(B, G)
    if G * Bg > B:
        xc = np.concatenate(
            [xc, np.zeros((G * Bg - B, T, I), xc.dtype)], axis=0
        )
    return np.ascontiguousarray(
        xc.reshape(G, Bg, T // NB, NB, I).transpose(0, 2, 4, 3, 1)
    ).astype(np.float16)


def run_gru(x, Wz, bz, Wr, br, Wh, bh, Wfc, bfc, n_cores=N_CORES, G=3,
            **spmd_kwargs):
    B_total, T, _ = x.shape
    B = B_total // n_cores
    nc = build_gru_nc(B, T, G=G)
    wmap = prep_weights(Wz, bz, Wr, br, Wh, bh, Wfc, bfc)
    in_maps = []
    for c in range(n_cores):
        xc = np.asarray(x[c * B : (c + 1) * B])
        in_maps.append({"xS": prep_x(xc, G), **wmap})
    res = run_bass_kernel_spmd(
        nc, in_maps, core_ids=list(range(n_cores)), **spmd_kwargs
    )
    y = np.concatenate(
        [res.results[c]["y"].T[:B] for c in range(n_cores)], axis=0
    ).astype(np.float32)
    return y, res


def kernel(x, Wz, bz, Wr, br, Wh, bh, Wfc, bfc):
    y, _ = run_gru(x, Wz, bz, Wr, br, Wh, bh, Wfc, bfc)
    return y


def make_inputs_for_timing(B, T, rng, G=3):
    """Random per-core input map matching build_gru_nc's dram tensors
    (timing only; values don't matter)."""
    Wz = rng.standard_normal((KC, H), dtype=np.float32) * 0.15
    Wr = rng.standard_normal((KC, H), dtype=np.float32) * 0.15
    Wh = rng.standard_normal((KC, H), dtype=np.float32) * 0.15
    Wfc = rng.standard_normal((H, HOR), dtype=np.float32) * 0.17
    wmap = prep_weights(
        Wz, rng.standard_normal(H, dtype=np.float32) * 0.15,
        Wr, rng.standard_normal(H, dtype=np.float32) * 0.15,
        Wh, rng.standard_normal(H, dtype=np.float32) * 0.15,
        Wfc, rng.standard_normal(HOR, dtype=np.float32) * 0.17,
    )
    xc = rng.standard_normal((B, T, I_IN), dtype=np.float32)
    return {"xS": prep_x(xc, G), **wmap}
